# revision 16
# baseline (speedup 1.0000x reference)
"""Deformable 3D conv (offset-predicting conv + trilinear-sampled 3x3x3 deform conv)
on 8 TRN2 NeuronCores.

Strategy: shard the output D axis (4 planes/core). Per core, fully pipelined
in 8 groups of 4 voxel-chunks (128 voxels each):
  1. Offset conv for the group as 27 shifted fp16 matmuls (PE).
  2. p-pipeline on DVE: clip, floor, fracs, row index r=(d*35+h)*35+w.
  3. Fold indices to the 16-partition-wrapped int16 layout dma_gather needs
     (DRAM bounce, split per group so it overlaps the gather stream).
  4. ONE dma_gather per chunk: 3456 samples x 512B, each spanning two
     256B table rows (w-corner pair trick halves the table).
  5. Trilinear lerp on DVE (w, d, h stages), fp16, frac factors pre-expanded
     on the Scalar (ACT) engine so DVE runs in 2x perf mode.
  6. Contraction over (n, c) as 7 accumulated PE matmuls -> out[64, v].
Table: one 256B row per padded voxel = 4 (d,h)-corners x 32 ch fp16,
row-contiguous in DRAM; built with 4 xbar DMA transposes.
"""
import os
import sys

for _p in ('/opt/trn_rl_repo', '/root/.axon_site/_ro/trn_rl_repo'):
    if os.path.isdir(_p) and _p not in sys.path:
        sys.path.insert(0, _p)

import numpy as np
import ml_dtypes  # noqa

import concourse.bass as bass
import concourse.mybir as mybir
import concourse.tile as tile
from concourse import bacc
from concourse.bass_utils import run_bass_kernel_spmd
from concourse.masks import make_identity

F32 = mybir.dt.float32
F16 = mybir.dt.float16
I32 = mybir.dt.int32
I16 = mybir.dt.int16
AL = mybir.AluOpType

# ---------------- problem constants ----------------
C = 32          # input channels
O = 64          # output channels
NN = 27         # kernel sample points
NCORES = 8
DSH = 4         # d-planes per core
V = DSH * 32 * 32   # voxels per core = 4096
P35 = 35
PL = 16         # XE d-planes per core
PLSZ = P35 * P35    # 1225
XE_ROWS = PL * PLSZ  # 19600
TROWS = 19712        # 154 * 128 (padded table rows)
GRPS = TROWS // 128  # 154
XE_FREE = 22400      # >= TROWS + max shift (1261)
NVC = 32             # voxel chunks of 128
NG = 8               # pipeline groups (4 chunks each)
NI = NN * 128        # gather indices per chunk

_PROGRAM_CACHE = {}


def _build_program():
    nc = bacc.Bacc("TRN2", target_bir_lowering=False, debug=False)

    xe_d = nc.dram_tensor("xe", [C, XE_FREE], F16, kind="ExternalInput").ap()
    pc_d = nc.dram_tensor("pc", [128, NVC * 96], F32, kind="ExternalInput").ap()  # per-group slices loaded on demand
    dclip_d = nc.dram_tensor("dclip", [128, 2], F32, kind="ExternalInput").ap()
    wp_d = nc.dram_tensor("wp", [32, NN * 96], F16, kind="ExternalInput").ap()
    wd_d = nc.dram_tensor("wd", [128, 7 * O], F16, kind="ExternalInput").ap()
    out_d = nc.dram_tensor("out_sl", [O, V], F32, kind="ExternalOutput").ap()
    t_d = nc.dram_tensor("tdram", [TROWS, 128], F16).ap()
    wrd = nc.dram_tensor("wrdram", [16, NVC, NN, 8], I16).ap()

    with tile.TileContext(nc) as tc:
        with tc.tile_pool(name="const", bufs=1) as cpool:
            ident = cpool.tile([128, 128], F32)
            make_identity(nc, ident[:])
            wp_sb = cpool.tile([32, NN * 96], F16)
            nc.sync.dma_start(wp_sb[:], wp_d)
            wd_sb = cpool.tile([128, 7 * O], F16)
            nc.sync.dma_start(wd_sb[:], wd_d)
            dclip_sb = cpool.tile([128, 2], F32)
            nc.sync.dma_start(dclip_sb[:], dclip_d)

            frac_t = cpool.tile([128, NVC * 96], F16)

            with tc.tile_pool(name="xe", bufs=1) as xepool:
                xe4 = xepool.tile([32, XE_FREE], F16)
                nc.sync.dma_start(xe4[:, :], xe_d)

                # ---------- corner-block table (4 dh-corners per 256B row) ----------
                with tc.tile_pool(name="tbl", bufs=1) as tpool:
                    t_sb = tpool.tile([128, GRPS * 128], F16, tag="tsb")
                    for ed in range(2):
                        for eh in range(2):
                            e = ed * 2 + eh
                            dlt = ed * PLSZ + eh * P35
                            src = xe4[0:32, dlt: dlt + GRPS * 128]
                            dst = t_sb[:].rearrange(
                                "p (g x) -> p g x", x=128
                            )[:, :, e * 32:(e + 1) * 32]
                            nc.sync.dma_start_transpose(out=dst, in_=src)
                    # DRAM row r = g*128 + p  (voxel-contiguous rows)
                    nc.sync.dma_start(
                        out=t_d.rearrange("(G p) x -> p G x", p=128),
                        in_=t_sb[:].rearrange("p (g x) -> p g x", x=128))

                with (
                    tc.tile_pool(name="convps", bufs=2, space="PSUM") as cps,
                    tc.tile_pool(name="trps", bufs=2, space="PSUM") as tps,
                    tc.tile_pool(name="ops", bufs=2, space="PSUM") as ops,
                    tc.tile_pool(name="pipe", bufs=2) as pipe,
                    tc.tile_pool(name="idxp", bufs=8) as idxp,
                    tc.tile_pool(name="wrp", bufs=8) as wrp,
                    tc.tile_pool(name="gat", bufs=3) as gpool,
                    tc.tile_pool(name="lerp", bufs=2) as lpool,
                    tc.tile_pool(name="fx", bufs=2) as fpool,
                    tc.tile_pool(name="accp", bufs=3) as apool,
                    tc.tile_pool(name="outp", bufs=4) as opool,
                ):
                    # ---------- head: conv + p-pipe per group ----------
                    idx16_groups = []
                    for g in range(NG):
                        dl, hh = g // 2, g % 2
                        psc = cps.tile([96, 512], F32, tag="convps")
                        for k in range(NN):
                            kd, kh, kw = k // 9, (k // 3) % 3, k % 3
                            b0 = (dl + kd + 5) * PLSZ + (hh * 16 + kh) * P35 + kw
                            rhs = xe4[:, b0:b0 + 16 * P35].rearrange(
                                "p (a b) -> p a b", b=P35)[:, :, 0:32]
                            nc.tensor.matmul(
                                psc[:, :],
                                lhsT=wp_sb[0:32, k * 96:(k + 1) * 96],
                                rhs=rhs,
                                start=(k == 0),
                                stop=(k == NN - 1),
                            )
                        offg = pipe.tile([96, 512], F32, tag="offg")
                        nc.scalar.copy(offg[:, :], psc[:, :])

                        c0 = g * 4 * 96
                        pcg = pipe.tile([128, 384], F32, tag="pcg")
                        nc.sync.dma_start(pcg[:], pc_d[:, c0:c0 + 384])
                        ptg = pipe.tile([128, 384], F32, tag="ptg")
                        for c4 in range(4):
                            ptp = tps.tile([128, 96], F32, tag="trps")
                            nc.tensor.transpose(
                                ptp[:, :],
                                offg[:, c4 * 128:(c4 + 1) * 128],
                                ident[0:96, 0:96],
                            )
                            nc.vector.tensor_add(
                                ptg[:, c4 * 96:(c4 + 1) * 96], ptp[:, :],
                                pcg[:, c4 * 96:(c4 + 1) * 96])

                        # p-pipeline for this group's 4 chunks (384 cols)
                        pg = ptg[:, :]
                        dv = pg.rearrange("p (v x) -> p v x", x=96)[:, :, 0:27]
                        hwv = pg.rearrange("p (v x) -> p v x", x=96)[:, :, 32:91]
                        nc.vector.scalar_tensor_tensor(
                            out=dv, in0=dv, scalar=dclip_sb[:, 0:1],
                            in1=dclip_sb[:, 1:2].rearrange(
                                "p (a b) -> p a b", b=1).to_broadcast((128, 4, 27)),
                            op0=AL.max, op1=AL.min)
                        nc.vector.tensor_scalar(
                            out=hwv, in0=hwv, scalar1=0.0, scalar2=33.0,
                            op0=AL.max, op1=AL.min)

                        q0i = pipe.tile([128, 384], I32, tag="q0i")
                        nc.vector.tensor_copy(q0i[:], pg)
                        q0f = pipe.tile([128, 384], F32, tag="q0f")
                        nc.vector.tensor_copy(q0f[:], q0i[:])
                        fixt = pipe.tile([128, 384], F32, tag="fixt")
                        nc.vector.tensor_tensor(out=fixt[:], in0=q0f[:], in1=pg,
                                                op=AL.is_gt)
                        nc.vector.tensor_sub(q0f[:], q0f[:], fixt[:])
                        nc.vector.tensor_sub(frac_t[:, c0:c0 + 384], pg, q0f[:])
                        q0dv = q0f[:].rearrange("p (v x) -> p v x", x=96)[:, :, 0:27]
                        nc.vector.tensor_scalar(
                            out=q0dv, in0=q0dv, scalar1=0.0, scalar2=14.0,
                            op0=AL.max, op1=AL.min)

                        q0hv = q0f[:].rearrange("p (v x) -> p v x", x=96)[:, :, 32:59]
                        q0wv = q0f[:].rearrange("p (v x) -> p v x", x=96)[:, :, 64:91]
                        idxf = pipe.tile([128, 4 * 27], F32, tag="idxf")
                        iv = idxf[:].rearrange("p (v x) -> p v x", x=27)
                        nc.vector.scalar_tensor_tensor(
                            out=iv, in0=q0dv, scalar=35.0, in1=q0hv,
                            op0=AL.mult, op1=AL.add)
                        nc.vector.scalar_tensor_tensor(
                            out=iv, in0=iv, scalar=35.0, in1=q0wv,
                            op0=AL.mult, op1=AL.add)
                        idx16 = idxp.tile([128, 4 * 27], I16, tag="idx16")
                        nc.vector.tensor_copy(idx16[:], idxf[:])

                        idx16_groups.append(idx16)

                    # ---------- main loop: gather + lerp + contract per chunk ----------
                    # ---------- idx fold per group (own tiles: no WAR with gathers) ----------
                    wr_groups = []
                    for g in range(NG):
                        idx16 = idx16_groups[g]
                        wrg = wrp.tile([128, 1024], I16, tag="wrg")
                        for r in range(8):
                            out_v = wrd[:, g * 4:(g + 1) * 4, :,
                                        r:r + 1].rearrange(
                                "q vc j u -> q vc (u j)")
                            in_v = idx16[16 * r:16 * (r + 1), :].rearrange(
                                "q (vc j) -> q vc j", vc=4, j=NN)
                            nc.sync.dma_start(out=out_v, in_=in_v)
                        nc.sync.dma_start(
                            out=wrg[0:16, :].rearrange(
                                "q (vc x) -> q vc x", x=256)[:, :, 0:NN * 8],
                            in_=wrd[:, g * 4:(g + 1) * 4, :, :].rearrange(
                                "q vc j r8 -> q vc (j r8)"))
                        nc.sync.dma_start(out=wrg[16:32, :], in_=wrg[0:16, :])
                        nc.sync.dma_start(out=wrg[32:64, :], in_=wrg[0:32, :])
                        nc.sync.dma_start(out=wrg[64:128, :], in_=wrg[0:64, :])
                        wr_groups.append(wrg)

                    gin_ap = bass.AP(t_d.tensor, 0, [[128, TROWS - 2], [1, 256]])
                    for vc in range(NVC):
                        rt = gpool.tile([128, NN * 256], F16, tag="rt")
                        nc.gpsimd.dma_gather(
                            out_ap=rt[:].rearrange("p (g x) -> p g x", x=256),
                            in_ap=gin_ap,
                            idxs_ap=wr_groups[vc // 4][
                                :, (vc % 4) * 256:(vc % 4) * 256 + NN * 8],
                            num_idxs=NI,
                            num_idxs_reg=NI,
                            elem_size=256,
                            elem_step=128,
                            single_packet=False,
                        )
                        rv = rt[:].rearrange("p (n x) -> p n x", x=256)

                        # expand frac factors to full width on ACT (2x DVE mode)
                        def _fexp(col, rep, tag):
                            fx = fpool.tile([128, NN * rep], F16, tag=tag)
                            fxv = fx[:].rearrange("p (n x) -> p n x", x=rep)
                            s = frac_t[:, vc * 96 + col: vc * 96 + col + 27]
                            nc.scalar.copy(
                                out=fxv,
                                in_=s.rearrange(
                                    "p (n o) -> p n o", o=1).to_broadcast(
                                    (128, NN, rep)))
                            return fxv
                        fw = _fexp(64, 128, "fw")
                        fd = _fexp(0, 64, "fd")
                        fh = _fexp(32, 32, "fh")

                        # rt row layout: [w-pair(2) x ed(2) x eh(2) x c(32)]
                        d1 = lpool.tile([128, NN * 128], F16, tag="d1")
                        av = d1[:].rearrange("p (n x) -> p n x", x=128)
                        nc.vector.tensor_sub(av, rv[:, :, 128:256], rv[:, :, 0:128])
                        nc.vector.tensor_tensor(out=av, in0=av, in1=fw, op=AL.mult)
                        nc.vector.tensor_add(av, av, rv[:, :, 0:128])

                        b1 = lpool.tile([128, NN * 64], F16, tag="b1")
                        bv = b1[:].rearrange("p (n x) -> p n x", x=64)
                        nc.vector.tensor_sub(bv, av[:, :, 64:128], av[:, :, 0:64])
                        nc.vector.tensor_tensor(out=bv, in0=bv, in1=fd, op=AL.mult)
                        nc.vector.tensor_add(bv, bv, av[:, :, 0:64])

                        acc = apool.tile([128, 896], F16, tag="acc")
                        nc.vector.memset(acc[:, NN * 32:896], 0.0)
                        cv = acc[:, 0:NN * 32].rearrange("p (n x) -> p n x", x=32)
                        nc.vector.tensor_sub(cv, bv[:, :, 32:64], bv[:, :, 0:32])
                        nc.vector.tensor_tensor(out=cv, in0=cv, in1=fh, op=AL.mult)
                        nc.vector.tensor_add(cv, cv, bv[:, :, 0:32])

                        acct = gpool.tile([128, 7, 128], F16, tag="acct")
                        nc.sync.dma_start_transpose(out=acct[:], in_=acc[:, :])

                        pso = ops.tile([64, 128], F32, tag="pso")
                        for gg in range(7):
                            nc.tensor.matmul(
                                pso[:, :],
                                lhsT=wd_sb[:, gg * O:(gg + 1) * O],
                                rhs=acct[:, gg, :],
                                start=(gg == 0), stop=(gg == 6))
                        osb = opool.tile([64, 128], F32, tag="osb")
                        nc.vector.tensor_copy(osb[:], pso[:, :])
                        nc.sync.dma_start(
                            out=out_d[:, vc * 128:(vc + 1) * 128], in_=osb[:])

    nc.compile()
    return nc


def _host_prep(x, w_p, b_p, w_d):
    """Build per-core input maps."""
    x = np.asarray(x, np.float32)
    w_p = np.asarray(w_p, np.float32)
    b_p = np.asarray(b_p, np.float32)
    w_d = np.asarray(w_d, np.float32)

    # global padded/extended volume, channel-first, fp16:
    # XG[c, g, h', w'] with g = xp_plane + 5 (xp planes -5..39), h', w' in [0,35)
    XG = np.zeros((C, 45, P35, P35), np.float16)
    XG[:, 6:38, 1:33, 1:33] = x[0].astype(np.float16)

    # pc (shared): [128, 32*96] f32
    v = np.arange(V)
    dl, hh, wl = v >> 10, (v >> 5) & 31, v & 31
    r = np.array([-1.0, 0.0, 1.0], np.float32)
    pn_d, pn_h, pn_w = np.meshgrid(r, r, r, indexing='ij')
    pn = np.stack([pn_d.ravel(), pn_h.ravel(), pn_w.ravel()])  # (3, 27)
    pc = np.zeros((V, 96), np.float32)
    pc[:, 0:27] = (dl[:, None] + 6.0) + pn[0][None, :] + b_p[None, 0:27]
    pc[:, 32:59] = (hh[:, None] + 1.0) + pn[1][None, :] + b_p[None, 27:54]
    pc[:, 64:91] = (wl[:, None] + 1.0) + pn[2][None, :] + b_p[None, 54:81]
    pc_t = pc.reshape(NVC, 128, 96).transpose(1, 0, 2).reshape(128, NVC * 96)
    pc_t = np.ascontiguousarray(pc_t, np.float32)

    # wp lhsT: [32, 27*96] fp16 (one 96-col slice per kernel tap)
    wp_l = np.zeros((32, NN * 96), np.float16)
    colmap = np.full(96, -1, np.int64)
    colmap[0:27] = np.arange(27)
    colmap[32:59] = 27 + np.arange(27)
    colmap[64:91] = 54 + np.arange(27)
    for k in range(NN):
        kd, kh, kw = k // 9, (k // 3) % 3, k % 3
        for m in range(96):
            ch = colmap[m]
            if ch < 0:
                continue
            wp_l[:, k * 96 + m] = w_p[ch, :, kd, kh, kw]

    # wd lhsT: [128, 7*64] fp16. K-row layout must match acc cols (n*32+c):
    # K = g*128 + pk -> n = (g*128+pk)//32, c = pk%32
    wd_l = np.zeros((128, 7 * O), np.float16)
    for g in range(7):
        for pk in range(128):
            n = 4 * g + pk // 32
            if n >= NN:
                continue
            wd_l[pk, g * O:(g + 1) * O] = w_d[:, pk % 32, n // 9, (n // 3) % 3, n % 3]

    in_maps = []
    for k in range(NCORES):
        dlo = 4 * k - 5
        xe = np.zeros((C, XE_FREE), np.float16)
        xe[:, :XE_ROWS] = XG[:, 4 * k:4 * k + PL].reshape(C, XE_ROWS)
        dclip = np.zeros((128, 2), np.float32)
        dclip[:, 0] = 0.0 - dlo
        dclip[:, 1] = 33.0 - dlo
        in_maps.append({
            "xe": xe,
            "pc": pc_t,
            "dclip": dclip,
            "wp": wp_l,
            "wd": wd_l,
        })
    return in_maps


def kernel(x, w_p, b_p, w_d):
    if "nc" not in _PROGRAM_CACHE:
        _PROGRAM_CACHE["nc"] = _build_program()
    nc = _PROGRAM_CACHE["nc"]
    in_maps = _host_prep(x, w_p, b_p, w_d)
    res = run_bass_kernel_spmd(nc, in_maps, list(range(NCORES))).results
    out = np.empty((1, O, 32, 32, 32), np.float32)
    for k in range(NCORES):
        out[0, :, 4 * k:4 * k + 4] = res[k]["out_sl"].reshape(O, DSH, 32, 32)
    return out


# revision 17
# speedup vs baseline: 1.2674x; 1.2674x over previous
"""Deformable 3D conv (offset-predicting conv + trilinear-sampled 3x3x3 deform conv)
on 8 TRN2 NeuronCores.

Strategy: shard the output D axis (4 planes/core). Per core, fully pipelined
in 8 groups of 4 voxel-chunks (128 voxels each):
  1. Offset conv for the group as 27 shifted fp16 matmuls (PE).
  2. p-pipeline on DVE: clip, floor, fracs, row index r=(d*35+h)*35+w.
  3. Fold indices to the 16-partition-wrapped int16 layout dma_gather needs
     (DRAM bounce, split per group so it overlaps the gather stream).
  4. ONE dma_gather per chunk: 3456 samples x 512B, each spanning two
     256B table rows (w-corner pair trick halves the table).
  5. Trilinear lerp on DVE (w, d, h stages), fp16, frac factors pre-expanded
     on the Scalar (ACT) engine so DVE runs in 2x perf mode.
  6. Contraction over (n, c) as 7 accumulated PE matmuls -> out[64, v].
Table: one 256B row per padded voxel = 4 (d,h)-corners x 32 ch fp16,
row-contiguous in DRAM; built with 4 xbar DMA transposes.
"""
import os
import sys

for _p in ('/opt/trn_rl_repo', '/root/.axon_site/_ro/trn_rl_repo'):
    if os.path.isdir(_p) and _p not in sys.path:
        sys.path.insert(0, _p)

import numpy as np
import ml_dtypes  # noqa

import concourse.bass as bass
import concourse.mybir as mybir
import concourse.tile as tile
from concourse import bacc
from concourse.bass_utils import run_bass_kernel_spmd
from concourse.masks import make_identity

F32 = mybir.dt.float32
F16 = mybir.dt.float16
I32 = mybir.dt.int32
I16 = mybir.dt.int16
AL = mybir.AluOpType

# ---------------- problem constants ----------------
C = 32          # input channels
O = 64          # output channels
NN = 27         # kernel sample points
NCORES = 8
DSH = 4         # d-planes per core
V = DSH * 32 * 32   # voxels per core = 4096
P35 = 35
PL = 16         # XE d-planes per core
PLSZ = P35 * P35    # 1225
XE_ROWS = PL * PLSZ  # 19600
TROWS = 19712        # 154 * 128 (padded table rows)
GRPS = TROWS // 128  # 154
XE_FREE = 22400      # >= TROWS + max shift (1261)
NVC = 32             # voxel chunks of 128
NG = 8               # pipeline groups (4 chunks each)
NI = NN * 128        # gather indices per chunk

_PROGRAM_CACHE = {}


def _build_program():
    nc = bacc.Bacc("TRN2", target_bir_lowering=False, debug=False)

    xe_d = nc.dram_tensor("xe", [C, XE_FREE], F16, kind="ExternalInput").ap()
    pc_d = nc.dram_tensor("pc", [128, NVC * 96], F32, kind="ExternalInput").ap()
    dclip_d = nc.dram_tensor("dclip", [128, 2], F32, kind="ExternalInput").ap()
    wp_d = nc.dram_tensor("wp", [32, NN * 96], F16, kind="ExternalInput").ap()
    wd_d = nc.dram_tensor("wd", [128, 7 * O], F16, kind="ExternalInput").ap()
    out_d = nc.dram_tensor("out_sl", [O, V], F32, kind="ExternalOutput").ap()
    t_d = nc.dram_tensor("tdram", [TROWS, 128], F16).ap()
    wrd = nc.dram_tensor("wrdram", [16, NVC, NN, 8], I16).ap()

    with tile.TileContext(nc) as tc:
        with tc.tile_pool(name="const", bufs=1) as cpool:
            ident = cpool.tile([128, 128], F32)
            make_identity(nc, ident[:])
            wp_sb = cpool.tile([32, NN * 96], F16)
            nc.sync.dma_start(wp_sb[:], wp_d)
            wd_sb = cpool.tile([128, 7 * O], F16)
            nc.sync.dma_start(wd_sb[:], wd_d)
            pc_sb = cpool.tile([128, NVC * 96], F32)
            nc.sync.dma_start(pc_sb[:], pc_d)
            dclip_sb = cpool.tile([128, 2], F32)
            nc.sync.dma_start(dclip_sb[:], dclip_d)

            frac_t = cpool.tile([128, NVC * 96], F16)
            wr16 = cpool.tile([128, NVC * 256], I16)
            p_t = cpool.tile([128, NVC * 96], F32)

            with tc.tile_pool(name="xe", bufs=1) as xepool:
                xe4 = xepool.tile([32, XE_FREE], F16)
                nc.sync.dma_start(xe4[:, :], xe_d)

                # ---------- corner-block table (4 dh-corners per 256B row) ----------
                with tc.tile_pool(name="tbl", bufs=1) as tpool:
                    t_sb = tpool.tile([128, GRPS * 128], F16, tag="tsb")
                    for ed in range(2):
                        for eh in range(2):
                            e = ed * 2 + eh
                            dlt = ed * PLSZ + eh * P35
                            src = xe4[0:32, dlt: dlt + GRPS * 128]
                            dst = t_sb[:].rearrange(
                                "p (g x) -> p g x", x=128
                            )[:, :, e * 32:(e + 1) * 32]
                            nc.sync.dma_start_transpose(out=dst, in_=src)
                    # DRAM row r = g*128 + p  (voxel-contiguous rows)
                    nc.sync.dma_start(
                        out=t_d.rearrange("(G p) x -> p G x", p=128),
                        in_=t_sb[:].rearrange("p (g x) -> p g x", x=128))

                with (
                    tc.tile_pool(name="convps", bufs=2, space="PSUM") as cps,
                    tc.tile_pool(name="trps", bufs=2, space="PSUM") as tps,
                    tc.tile_pool(name="ops", bufs=2, space="PSUM") as ops,
                    tc.tile_pool(name="pipe", bufs=2) as pipe,
                    tc.tile_pool(name="gat", bufs=3) as gpool,
                    tc.tile_pool(name="lerp", bufs=2) as lpool,
                    tc.tile_pool(name="fx", bufs=2) as fpool,
                    tc.tile_pool(name="accp", bufs=3) as apool,
                    tc.tile_pool(name="outp", bufs=4) as opool,
                ):
                    # ---------- head: conv + p-pipe + idx fold, per group ----------
                    for g in range(NG):
                        dl, hh = g // 2, g % 2
                        psc = cps.tile([96, 512], F32, tag="convps")
                        for k in range(NN):
                            kd, kh, kw = k // 9, (k // 3) % 3, k % 3
                            b0 = (dl + kd + 5) * PLSZ + (hh * 16 + kh) * P35 + kw
                            rhs = xe4[:, b0:b0 + 16 * P35].rearrange(
                                "p (a b) -> p a b", b=P35)[:, :, 0:32]
                            nc.tensor.matmul(
                                psc[:, :],
                                lhsT=wp_sb[0:32, k * 96:(k + 1) * 96],
                                rhs=rhs,
                                start=(k == 0),
                                stop=(k == NN - 1),
                            )
                        offg = pipe.tile([96, 512], F32, tag="offg")
                        nc.scalar.copy(offg[:, :], psc[:, :])

                        for c4 in range(4):
                            ch = g * 4 + c4
                            ptp = tps.tile([128, 96], F32, tag="trps")
                            nc.tensor.transpose(
                                ptp[:, :],
                                offg[:, c4 * 128:(c4 + 1) * 128],
                                ident[0:96, 0:96],
                            )
                            nc.vector.tensor_add(
                                p_t[:, ch * 96:(ch + 1) * 96], ptp[:, :],
                                pc_sb[:, ch * 96:(ch + 1) * 96])

                        # p-pipeline for this group's 4 chunks (384 cols)
                        c0 = g * 4 * 96
                        pg = p_t[:, c0:c0 + 384]
                        dv = pg.rearrange("p (v x) -> p v x", x=96)[:, :, 0:27]
                        hwv = pg.rearrange("p (v x) -> p v x", x=96)[:, :, 32:91]
                        nc.vector.scalar_tensor_tensor(
                            out=dv, in0=dv, scalar=dclip_sb[:, 0:1],
                            in1=dclip_sb[:, 1:2].rearrange(
                                "p (a b) -> p a b", b=1).to_broadcast((128, 4, 27)),
                            op0=AL.max, op1=AL.min)
                        nc.vector.tensor_scalar(
                            out=hwv, in0=hwv, scalar1=0.0, scalar2=33.0,
                            op0=AL.max, op1=AL.min)

                        q0i = pipe.tile([128, 384], I32, tag="q0i")
                        nc.vector.tensor_copy(q0i[:], pg)
                        q0f = pipe.tile([128, 384], F32, tag="q0f")
                        nc.vector.tensor_copy(q0f[:], q0i[:])
                        fixt = pipe.tile([128, 384], F32, tag="fixt")
                        nc.vector.tensor_tensor(out=fixt[:], in0=q0f[:], in1=pg,
                                                op=AL.is_gt)
                        nc.vector.tensor_sub(q0f[:], q0f[:], fixt[:])
                        nc.vector.tensor_sub(frac_t[:, c0:c0 + 384], pg, q0f[:])
                        q0dv = q0f[:].rearrange("p (v x) -> p v x", x=96)[:, :, 0:27]
                        nc.vector.tensor_scalar(
                            out=q0dv, in0=q0dv, scalar1=0.0, scalar2=14.0,
                            op0=AL.max, op1=AL.min)

                        q0hv = q0f[:].rearrange("p (v x) -> p v x", x=96)[:, :, 32:59]
                        q0wv = q0f[:].rearrange("p (v x) -> p v x", x=96)[:, :, 64:91]
                        idxf = pipe.tile([128, 4 * 27], F32, tag="idxf")
                        iv = idxf[:].rearrange("p (v x) -> p v x", x=27)
                        nc.vector.scalar_tensor_tensor(
                            out=iv, in0=q0dv, scalar=35.0, in1=q0hv,
                            op0=AL.mult, op1=AL.add)
                        nc.vector.scalar_tensor_tensor(
                            out=iv, in0=iv, scalar=35.0, in1=q0wv,
                            op0=AL.mult, op1=AL.add)
                        idx16 = pipe.tile([128, 4 * 27], I16, tag="idx16")
                        nc.vector.tensor_copy(idx16[:], idxf[:])

                        # fold to wrapped layout via DRAM bounce (per group):
                        # wr16[q, vc, 8j + r] = idx16[16r + q, vc, j]
                        for r in range(8):
                            out_v = wrd[:, g * 4:(g + 1) * 4, :, r:r + 1].rearrange(
                                "q vc j u -> q vc (u j)")
                            in_v = idx16[16 * r:16 * (r + 1), :].rearrange(
                                "q (vc j) -> q vc j", vc=4, j=NN)
                            nc.sync.dma_start(out=out_v, in_=in_v)
                        gcol = g * 4 * 256
                        nc.sync.dma_start(
                            out=wr16[0:16, gcol:gcol + 1024].rearrange(
                                "q (vc x) -> q vc x", x=256)[:, :, 0:NN * 8],
                            in_=wrd[:, g * 4:(g + 1) * 4, :, :].rearrange(
                                "q vc j r8 -> q vc (j r8)"))
                        nc.sync.dma_start(out=wr16[16:32, gcol:gcol + 1024],
                                          in_=wr16[0:16, gcol:gcol + 1024])
                        nc.sync.dma_start(out=wr16[32:64, gcol:gcol + 1024],
                                          in_=wr16[0:32, gcol:gcol + 1024])
                        nc.sync.dma_start(out=wr16[64:128, gcol:gcol + 1024],
                                          in_=wr16[0:64, gcol:gcol + 1024])

                    # ---------- main loop: gather + lerp + contract per chunk ----------
                    gin_ap = bass.AP(t_d.tensor, 0, [[128, TROWS - 2], [1, 256]])
                    for vc in range(NVC):
                        rt = gpool.tile([128, NN * 256], F16, tag="rt")
                        nc.gpsimd.dma_gather(
                            out_ap=rt[:].rearrange("p (g x) -> p g x", x=256),
                            in_ap=gin_ap,
                            idxs_ap=wr16[:, vc * 256:vc * 256 + NN * 8],
                            num_idxs=NI,
                            num_idxs_reg=NI,
                            elem_size=256,
                            elem_step=128,
                            single_packet=False,
                        )
                        rv = rt[:].rearrange("p (n x) -> p n x", x=256)

                        # expand frac factors to full width on ACT (2x DVE mode)
                        def _fexp(col, rep, tag):
                            fx = fpool.tile([128, NN * rep], F16, tag=tag)
                            fxv = fx[:].rearrange("p (n x) -> p n x", x=rep)
                            s = frac_t[:, vc * 96 + col: vc * 96 + col + 27]
                            nc.scalar.copy(
                                out=fxv,
                                in_=s.rearrange(
                                    "p (n o) -> p n o", o=1).to_broadcast(
                                    (128, NN, rep)))
                            return fxv
                        fw = _fexp(64, 128, "fw")
                        fd = _fexp(0, 64, "fd")
                        fh = _fexp(32, 32, "fh")

                        # rt row layout: [w-pair(2) x ed(2) x eh(2) x c(32)]
                        d1 = lpool.tile([128, NN * 128], F16, tag="d1")
                        av = d1[:].rearrange("p (n x) -> p n x", x=128)
                        nc.vector.tensor_sub(av, rv[:, :, 128:256], rv[:, :, 0:128])
                        nc.vector.tensor_tensor(out=av, in0=av, in1=fw, op=AL.mult)
                        nc.vector.tensor_add(av, av, rv[:, :, 0:128])

                        b1 = lpool.tile([128, NN * 64], F16, tag="b1")
                        bv = b1[:].rearrange("p (n x) -> p n x", x=64)
                        nc.vector.tensor_sub(bv, av[:, :, 64:128], av[:, :, 0:64])
                        nc.vector.tensor_tensor(out=bv, in0=bv, in1=fd, op=AL.mult)
                        nc.vector.tensor_add(bv, bv, av[:, :, 0:64])

                        acc = apool.tile([128, 896], F16, tag="acc")
                        nc.vector.memset(acc[:, NN * 32:896], 0.0)
                        cv = acc[:, 0:NN * 32].rearrange("p (n x) -> p n x", x=32)
                        nc.vector.tensor_sub(cv, bv[:, :, 32:64], bv[:, :, 0:32])
                        nc.vector.tensor_tensor(out=cv, in0=cv, in1=fh, op=AL.mult)
                        nc.vector.tensor_add(cv, cv, bv[:, :, 0:32])

                        acct = gpool.tile([128, 7, 128], F16, tag="acct")
                        nc.sync.dma_start_transpose(out=acct[:], in_=acc[:, :])

                        pso = ops.tile([64, 128], F32, tag="pso")
                        for gg in range(7):
                            nc.tensor.matmul(
                                pso[:, :],
                                lhsT=wd_sb[:, gg * O:(gg + 1) * O],
                                rhs=acct[:, gg, :],
                                start=(gg == 0), stop=(gg == 6))
                        osb = opool.tile([64, 128], F32, tag="osb")
                        nc.scalar.copy(osb[:], pso[:, :])
                        nc.sync.dma_start(
                            out=out_d[:, vc * 128:(vc + 1) * 128], in_=osb[:])

    nc.compile()
    return nc


def _host_prep(x, w_p, b_p, w_d):
    """Build per-core input maps."""
    x = np.asarray(x, np.float32)
    w_p = np.asarray(w_p, np.float32)
    b_p = np.asarray(b_p, np.float32)
    w_d = np.asarray(w_d, np.float32)

    # global padded/extended volume, channel-first, fp16:
    # XG[c, g, h', w'] with g = xp_plane + 5 (xp planes -5..39), h', w' in [0,35)
    XG = np.zeros((C, 45, P35, P35), np.float16)
    XG[:, 6:38, 1:33, 1:33] = x[0].astype(np.float16)

    # pc (shared): [128, 32*96] f32
    v = np.arange(V)
    dl, hh, wl = v >> 10, (v >> 5) & 31, v & 31
    r = np.array([-1.0, 0.0, 1.0], np.float32)
    pn_d, pn_h, pn_w = np.meshgrid(r, r, r, indexing='ij')
    pn = np.stack([pn_d.ravel(), pn_h.ravel(), pn_w.ravel()])  # (3, 27)
    pc = np.zeros((V, 96), np.float32)
    pc[:, 0:27] = (dl[:, None] + 6.0) + pn[0][None, :] + b_p[None, 0:27]
    pc[:, 32:59] = (hh[:, None] + 1.0) + pn[1][None, :] + b_p[None, 27:54]
    pc[:, 64:91] = (wl[:, None] + 1.0) + pn[2][None, :] + b_p[None, 54:81]
    pc_t = pc.reshape(NVC, 128, 96).transpose(1, 0, 2).reshape(128, NVC * 96)
    pc_t = np.ascontiguousarray(pc_t, np.float32)

    # wp lhsT: [32, 27*96] fp16 (one 96-col slice per kernel tap)
    wp_l = np.zeros((32, NN * 96), np.float16)
    colmap = np.full(96, -1, np.int64)
    colmap[0:27] = np.arange(27)
    colmap[32:59] = 27 + np.arange(27)
    colmap[64:91] = 54 + np.arange(27)
    for k in range(NN):
        kd, kh, kw = k // 9, (k // 3) % 3, k % 3
        for m in range(96):
            ch = colmap[m]
            if ch < 0:
                continue
            wp_l[:, k * 96 + m] = w_p[ch, :, kd, kh, kw]

    # wd lhsT: [128, 7*64] fp16. K-row layout must match acc cols (n*32+c):
    # K = g*128 + pk -> n = (g*128+pk)//32, c = pk%32
    wd_l = np.zeros((128, 7 * O), np.float16)
    for g in range(7):
        for pk in range(128):
            n = 4 * g + pk // 32
            if n >= NN:
                continue
            wd_l[pk, g * O:(g + 1) * O] = w_d[:, pk % 32, n // 9, (n // 3) % 3, n % 3]

    in_maps = []
    for k in range(NCORES):
        dlo = 4 * k - 5
        xe = np.zeros((C, XE_FREE), np.float16)
        xe[:, :XE_ROWS] = XG[:, 4 * k:4 * k + PL].reshape(C, XE_ROWS)
        dclip = np.zeros((128, 2), np.float32)
        dclip[:, 0] = 0.0 - dlo
        dclip[:, 1] = 33.0 - dlo
        in_maps.append({
            "xe": xe,
            "pc": pc_t,
            "dclip": dclip,
            "wp": wp_l,
            "wd": wd_l,
        })
    return in_maps


def kernel(x, w_p, b_p, w_d):
    if "nc" not in _PROGRAM_CACHE:
        _PROGRAM_CACHE["nc"] = _build_program()
    nc = _PROGRAM_CACHE["nc"]
    in_maps = _host_prep(x, w_p, b_p, w_d)
    res = run_bass_kernel_spmd(nc, in_maps, list(range(NCORES))).results
    out = np.empty((1, O, 32, 32, 32), np.float32)
    for k in range(NCORES):
        out[0, :, 4 * k:4 * k + 4] = res[k]["out_sl"].reshape(O, DSH, 32, 32)
    return out


# revision 19
# speedup vs baseline: 1.3807x; 1.0894x over previous
"""Deformable 3D conv (offset-predicting conv + trilinear-sampled 3x3x3 deform conv)
on 8 TRN2 NeuronCores.

Strategy: shard the output D axis (4 planes/core). Per core, fully pipelined
in 8 groups of 4 voxel-chunks (128 voxels each):
  1. Offset conv for the group as 27 shifted fp16 matmuls (PE).
  2. p-pipeline on DVE: clip, floor, fracs, row index r=(d*35+h)*35+w.
  3. Fold indices to the 16-partition-wrapped int16 layout dma_gather needs
     (DRAM bounce, split per group so it overlaps the gather stream).
  4. ONE dma_gather per chunk: 3456 samples x 512B, each spanning two
     256B table rows (w-corner pair trick halves the table).
  5. Trilinear lerp on DVE (w, d, h stages), fp16, frac factors pre-expanded
     on the Scalar (ACT) engine so DVE runs in 2x perf mode.
  6. Contraction over (n, c) as 7 accumulated PE matmuls -> out[64, v].
Table: one 256B row per padded voxel = 4 (d,h)-corners x 32 ch fp16,
row-contiguous in DRAM; built with 4 xbar DMA transposes.
"""
import os
import sys

for _p in ('/opt/trn_rl_repo', '/root/.axon_site/_ro/trn_rl_repo'):
    if os.path.isdir(_p) and _p not in sys.path:
        sys.path.insert(0, _p)

import numpy as np
import ml_dtypes  # noqa

import concourse.bass as bass
import concourse.mybir as mybir
import concourse.tile as tile
from concourse import bacc
from concourse.bass_utils import run_bass_kernel_spmd
from concourse.masks import make_identity

F32 = mybir.dt.float32
F16 = mybir.dt.float16
I32 = mybir.dt.int32
I16 = mybir.dt.int16
AL = mybir.AluOpType

# ---------------- problem constants ----------------
C = 32          # input channels
O = 64          # output channels
NN = 27         # kernel sample points
NCORES = 8
DSH = 4         # d-planes per core
V = DSH * 32 * 32   # voxels per core = 4096
P35 = 35
PL = 16         # XE d-planes per core
PLSZ = P35 * P35    # 1225
XE_ROWS = PL * PLSZ  # 19600
TROWS = 19712        # 154 * 128 (padded table rows)
GRPS = TROWS // 128  # 154
XE_FREE = 22400      # >= TROWS + max shift (1261)
NVC = 32             # voxel chunks of 128
NG = 8               # pipeline groups (4 chunks each)
NI = NN * 128        # gather indices per chunk

_PROGRAM_CACHE = {}


def _build_program():
    nc = bacc.Bacc("TRN2", target_bir_lowering=False, debug=False)

    xe_d = nc.dram_tensor("xe", [C, XE_FREE], F16, kind="ExternalInput").ap()
    pc_d = nc.dram_tensor("pc", [128, NVC * 96], F32, kind="ExternalInput").ap()
    dclip_d = nc.dram_tensor("dclip", [128, 2], F32, kind="ExternalInput").ap()
    wp_d = nc.dram_tensor("wp", [32, NN * 96], F16, kind="ExternalInput").ap()
    wd_d = nc.dram_tensor("wd", [128, 7 * O], F16, kind="ExternalInput").ap()
    out_d = nc.dram_tensor("out_sl", [O, V], F32, kind="ExternalOutput").ap()
    t_d = nc.dram_tensor("tdram", [TROWS, 128], F16).ap()
    wrd = nc.dram_tensor("wrdram", [16, NVC, NN, 8], I16).ap()

    with tile.TileContext(nc) as tc:
        with tc.tile_pool(name="const", bufs=1) as cpool:
            ident = cpool.tile([128, 128], F32)
            make_identity(nc, ident[:])
            wp_sb = cpool.tile([32, NN * 96], F16)
            nc.sync.dma_start(wp_sb[:], wp_d)
            wd_sb = cpool.tile([128, 7 * O], F16)
            nc.sync.dma_start(wd_sb[:], wd_d)
            pc_sb = cpool.tile([128, NVC * 96], F32)
            nc.sync.dma_start(pc_sb[:], pc_d)
            dclip_sb = cpool.tile([128, 2], F32)
            nc.sync.dma_start(dclip_sb[:], dclip_d)

            frac_t = cpool.tile([128, NVC * 96], F16)
            p_t = cpool.tile([128, NVC * 96], F32)
            wr_groups = []
            for _wi in range(NG):
                _t = cpool.tile([128, 1024], I16, tag=f"wrg{_wi}")
                wr_groups.append(_t)

            with tc.tile_pool(name="xe", bufs=1) as xepool:
                xe4 = xepool.tile([32, XE_FREE], F16)
                nc.sync.dma_start(xe4[:, :], xe_d)

                # ---------- corner-block table (4 dh-corners per 256B row) ----------
                with tc.tile_pool(name="tbl", bufs=1) as tpool:
                    t_sb = tpool.tile([128, GRPS * 128], F16, tag="tsb")
                    for ed in range(2):
                        for eh in range(2):
                            e = ed * 2 + eh
                            dlt = ed * PLSZ + eh * P35
                            src = xe4[0:32, dlt: dlt + GRPS * 128]
                            dst = t_sb[:].rearrange(
                                "p (g x) -> p g x", x=128
                            )[:, :, e * 32:(e + 1) * 32]
                            nc.sync.dma_start_transpose(out=dst, in_=src)
                    # DRAM row r = g*128 + p  (voxel-contiguous rows)
                    nc.sync.dma_start(
                        out=t_d.rearrange("(G p) x -> p G x", p=128),
                        in_=t_sb[:].rearrange("p (g x) -> p g x", x=128))

                with (
                    tc.tile_pool(name="convps", bufs=2, space="PSUM") as cps,
                    tc.tile_pool(name="trps", bufs=2, space="PSUM") as tps,
                    tc.tile_pool(name="ops", bufs=2, space="PSUM") as ops,
                    tc.tile_pool(name="pipe", bufs=2) as pipe,
                    tc.tile_pool(name="gat", bufs=3) as gpool,
                    tc.tile_pool(name="lerp", bufs=2) as lpool,
                    tc.tile_pool(name="fx", bufs=2) as fpool,
                    tc.tile_pool(name="accp", bufs=3) as apool,
                    tc.tile_pool(name="outp", bufs=4) as opool,
                ):
                    # ---------- head: conv + p-pipe + idx fold, per group ----------
                    for g in range(NG):
                        dl, hh = g // 2, g % 2
                        psc = cps.tile([96, 512], F32, tag="convps")
                        for k in range(NN):
                            kd, kh, kw = k // 9, (k // 3) % 3, k % 3
                            b0 = (dl + kd + 5) * PLSZ + (hh * 16 + kh) * P35 + kw
                            rhs = xe4[:, b0:b0 + 16 * P35].rearrange(
                                "p (a b) -> p a b", b=P35)[:, :, 0:32]
                            nc.tensor.matmul(
                                psc[:, :],
                                lhsT=wp_sb[0:32, k * 96:(k + 1) * 96],
                                rhs=rhs,
                                start=(k == 0),
                                stop=(k == NN - 1),
                            )
                        offg = pipe.tile([96, 512], F32, tag="offg")
                        nc.scalar.copy(offg[:, :], psc[:, :])

                        for c4 in range(4):
                            ch = g * 4 + c4
                            ptp = tps.tile([128, 96], F32, tag="trps")
                            nc.tensor.transpose(
                                ptp[:, :],
                                offg[:, c4 * 128:(c4 + 1) * 128],
                                ident[0:96, 0:96],
                            )
                            nc.vector.tensor_add(
                                p_t[:, ch * 96:(ch + 1) * 96], ptp[:, :],
                                pc_sb[:, ch * 96:(ch + 1) * 96])

                        # p-pipeline for this group's 4 chunks (384 cols)
                        c0 = g * 4 * 96
                        pg = p_t[:, c0:c0 + 384]
                        dv = pg.rearrange("p (v x) -> p v x", x=96)[:, :, 0:27]
                        hwv = pg.rearrange("p (v x) -> p v x", x=96)[:, :, 32:91]
                        nc.vector.scalar_tensor_tensor(
                            out=dv, in0=dv, scalar=dclip_sb[:, 0:1],
                            in1=dclip_sb[:, 1:2].rearrange(
                                "p (a b) -> p a b", b=1).to_broadcast((128, 4, 27)),
                            op0=AL.max, op1=AL.min)
                        nc.vector.tensor_scalar(
                            out=hwv, in0=hwv, scalar1=0.0, scalar2=33.0,
                            op0=AL.max, op1=AL.min)

                        q0i = pipe.tile([128, 384], I32, tag="q0i")
                        nc.vector.tensor_copy(q0i[:], pg)
                        q0f = pipe.tile([128, 384], F32, tag="q0f")
                        nc.vector.tensor_copy(q0f[:], q0i[:])
                        fixt = pipe.tile([128, 384], F32, tag="fixt")
                        nc.vector.tensor_tensor(out=fixt[:], in0=q0f[:], in1=pg,
                                                op=AL.is_gt)
                        nc.vector.tensor_sub(q0f[:], q0f[:], fixt[:])
                        nc.vector.tensor_sub(frac_t[:, c0:c0 + 384], pg, q0f[:])
                        q0dv = q0f[:].rearrange("p (v x) -> p v x", x=96)[:, :, 0:27]
                        nc.vector.tensor_scalar(
                            out=q0dv, in0=q0dv, scalar1=0.0, scalar2=14.0,
                            op0=AL.max, op1=AL.min)

                        q0hv = q0f[:].rearrange("p (v x) -> p v x", x=96)[:, :, 32:59]
                        q0wv = q0f[:].rearrange("p (v x) -> p v x", x=96)[:, :, 64:91]
                        idxf = pipe.tile([128, 4 * 27], F32, tag="idxf")
                        iv = idxf[:].rearrange("p (v x) -> p v x", x=27)
                        nc.vector.scalar_tensor_tensor(
                            out=iv, in0=q0dv, scalar=35.0, in1=q0hv,
                            op0=AL.mult, op1=AL.add)
                        nc.vector.scalar_tensor_tensor(
                            out=iv, in0=iv, scalar=35.0, in1=q0wv,
                            op0=AL.mult, op1=AL.add)
                        idx16 = pipe.tile([128, 4 * 27], I16, tag="idx16")
                        nc.vector.tensor_copy(idx16[:], idxf[:])

                        # fold to wrapped layout via DRAM bounce (per group):
                        # wrg[q, vc, 8j + r] = idx16[16r + q, vc, j]
                        wrg = wr_groups[g]
                        for r in range(8):
                            out_v = wrd[:, g * 4:(g + 1) * 4, :, r:r + 1].rearrange(
                                "q vc j u -> q vc (u j)")
                            in_v = idx16[16 * r:16 * (r + 1), :].rearrange(
                                "q (vc j) -> q vc j", vc=4, j=NN)
                            nc.sync.dma_start(out=out_v, in_=in_v)
                        nc.sync.dma_start(
                            out=wrg[0:16, :].rearrange(
                                "q (vc x) -> q vc x", x=256)[:, :, 0:NN * 8],
                            in_=wrd[:, g * 4:(g + 1) * 4, :, :].rearrange(
                                "q vc j r8 -> q vc (j r8)"))
                        nc.sync.dma_start(out=wrg[16:32, :], in_=wrg[0:16, :])
                        nc.sync.dma_start(out=wrg[32:64, :], in_=wrg[0:32, :])
                        nc.sync.dma_start(out=wrg[64:128, :], in_=wrg[0:64, :])

                    # ---------- main loop: gather + lerp + contract per chunk ----------
                    gin_ap = bass.AP(t_d.tensor, 0, [[128, TROWS - 2], [1, 256]])
                    for vc in range(NVC):
                        rt = gpool.tile([128, NN * 256], F16, tag="rt")
                        nc.gpsimd.dma_gather(
                            out_ap=rt[:].rearrange("p (g x) -> p g x", x=256),
                            in_ap=gin_ap,
                            idxs_ap=wr_groups[vc // 4][
                                :, (vc % 4) * 256:(vc % 4) * 256 + NN * 8],
                            num_idxs=NI,
                            num_idxs_reg=NI,
                            elem_size=256,
                            elem_step=128,
                            single_packet=False,
                        )
                        rv = rt[:].rearrange("p (n x) -> p n x", x=256)

                        # expand frac factors to full width on ACT (2x DVE mode)
                        def _fexp(col, rep, tag):
                            fx = fpool.tile([128, NN * rep], F16, tag=tag)
                            fxv = fx[:].rearrange("p (n x) -> p n x", x=rep)
                            s = frac_t[:, vc * 96 + col: vc * 96 + col + 27]
                            nc.scalar.copy(
                                out=fxv,
                                in_=s.rearrange(
                                    "p (n o) -> p n o", o=1).to_broadcast(
                                    (128, NN, rep)))
                            return fxv
                        fw = _fexp(64, 128, "fw")
                        fd = _fexp(0, 64, "fd")
                        fh = _fexp(32, 32, "fh")

                        # rt row layout: [w-pair(2) x ed(2) x eh(2) x c(32)]
                        d1 = lpool.tile([128, NN * 128], F16, tag="d1")
                        av = d1[:].rearrange("p (n x) -> p n x", x=128)
                        nc.vector.tensor_sub(av, rv[:, :, 128:256], rv[:, :, 0:128])
                        nc.vector.tensor_tensor(out=av, in0=av, in1=fw, op=AL.mult)
                        nc.vector.tensor_add(av, av, rv[:, :, 0:128])

                        b1 = lpool.tile([128, NN * 64], F16, tag="b1")
                        bv = b1[:].rearrange("p (n x) -> p n x", x=64)
                        nc.vector.tensor_sub(bv, av[:, :, 64:128], av[:, :, 0:64])
                        nc.vector.tensor_tensor(out=bv, in0=bv, in1=fd, op=AL.mult)
                        nc.vector.tensor_add(bv, bv, av[:, :, 0:64])

                        acc = apool.tile([128, 896], F16, tag="acc")
                        nc.vector.memset(acc[:, NN * 32:896], 0.0)
                        cv = acc[:, 0:NN * 32].rearrange("p (n x) -> p n x", x=32)
                        nc.vector.tensor_sub(cv, bv[:, :, 32:64], bv[:, :, 0:32])
                        nc.vector.tensor_tensor(out=cv, in0=cv, in1=fh, op=AL.mult)
                        nc.vector.tensor_add(cv, cv, bv[:, :, 0:32])

                        acct = gpool.tile([128, 7, 128], F16, tag="acct")
                        nc.sync.dma_start_transpose(out=acct[:], in_=acc[:, :])

                        pso = ops.tile([64, 128], F32, tag="pso")
                        for gg in range(7):
                            nc.tensor.matmul(
                                pso[:, :],
                                lhsT=wd_sb[:, gg * O:(gg + 1) * O],
                                rhs=acct[:, gg, :],
                                start=(gg == 0), stop=(gg == 6))
                        osb = opool.tile([64, 128], F32, tag="osb")
                        nc.scalar.copy(osb[:], pso[:, :])
                        nc.sync.dma_start(
                            out=out_d[:, vc * 128:(vc + 1) * 128], in_=osb[:])

    nc.compile()
    return nc


def _host_prep(x, w_p, b_p, w_d):
    """Build per-core input maps."""
    x = np.asarray(x, np.float32)
    w_p = np.asarray(w_p, np.float32)
    b_p = np.asarray(b_p, np.float32)
    w_d = np.asarray(w_d, np.float32)

    # global padded/extended volume, channel-first, fp16:
    # XG[c, g, h', w'] with g = xp_plane + 5 (xp planes -5..39), h', w' in [0,35)
    XG = np.zeros((C, 45, P35, P35), np.float16)
    XG[:, 6:38, 1:33, 1:33] = x[0].astype(np.float16)

    # pc (shared): [128, 32*96] f32
    v = np.arange(V)
    dl, hh, wl = v >> 10, (v >> 5) & 31, v & 31
    r = np.array([-1.0, 0.0, 1.0], np.float32)
    pn_d, pn_h, pn_w = np.meshgrid(r, r, r, indexing='ij')
    pn = np.stack([pn_d.ravel(), pn_h.ravel(), pn_w.ravel()])  # (3, 27)
    pc = np.zeros((V, 96), np.float32)
    pc[:, 0:27] = (dl[:, None] + 6.0) + pn[0][None, :] + b_p[None, 0:27]
    pc[:, 32:59] = (hh[:, None] + 1.0) + pn[1][None, :] + b_p[None, 27:54]
    pc[:, 64:91] = (wl[:, None] + 1.0) + pn[2][None, :] + b_p[None, 54:81]
    pc_t = pc.reshape(NVC, 128, 96).transpose(1, 0, 2).reshape(128, NVC * 96)
    pc_t = np.ascontiguousarray(pc_t, np.float32)

    # wp lhsT: [32, 27*96] fp16 (one 96-col slice per kernel tap)
    wp_l = np.zeros((32, NN * 96), np.float16)
    colmap = np.full(96, -1, np.int64)
    colmap[0:27] = np.arange(27)
    colmap[32:59] = 27 + np.arange(27)
    colmap[64:91] = 54 + np.arange(27)
    for k in range(NN):
        kd, kh, kw = k // 9, (k // 3) % 3, k % 3
        for m in range(96):
            ch = colmap[m]
            if ch < 0:
                continue
            wp_l[:, k * 96 + m] = w_p[ch, :, kd, kh, kw]

    # wd lhsT: [128, 7*64] fp16. K-row layout must match acc cols (n*32+c):
    # K = g*128 + pk -> n = (g*128+pk)//32, c = pk%32
    wd_l = np.zeros((128, 7 * O), np.float16)
    for g in range(7):
        for pk in range(128):
            n = 4 * g + pk // 32
            if n >= NN:
                continue
            wd_l[pk, g * O:(g + 1) * O] = w_d[:, pk % 32, n // 9, (n // 3) % 3, n % 3]

    in_maps = []
    for k in range(NCORES):
        dlo = 4 * k - 5
        xe = np.zeros((C, XE_FREE), np.float16)
        xe[:, :XE_ROWS] = XG[:, 4 * k:4 * k + PL].reshape(C, XE_ROWS)
        dclip = np.zeros((128, 2), np.float32)
        dclip[:, 0] = 0.0 - dlo
        dclip[:, 1] = 33.0 - dlo
        in_maps.append({
            "xe": xe,
            "pc": pc_t,
            "dclip": dclip,
            "wp": wp_l,
            "wd": wd_l,
        })
    return in_maps


def kernel(x, w_p, b_p, w_d):
    if "nc" not in _PROGRAM_CACHE:
        _PROGRAM_CACHE["nc"] = _build_program()
    nc = _PROGRAM_CACHE["nc"]
    in_maps = _host_prep(x, w_p, b_p, w_d)
    res = run_bass_kernel_spmd(nc, in_maps, list(range(NCORES))).results
    out = np.empty((1, O, 32, 32, 32), np.float32)
    for k in range(NCORES):
        out[0, :, 4 * k:4 * k + 4] = res[k]["out_sl"].reshape(O, DSH, 32, 32)
    return out


# revision 20
# speedup vs baseline: 1.4403x; 1.0432x over previous
"""Deformable 3D conv (offset-predicting conv + trilinear-sampled 3x3x3 deform conv)
on 8 TRN2 NeuronCores.

Strategy: shard the output D axis (4 planes/core). Per core, fully pipelined
in 8 groups of 4 voxel-chunks (128 voxels each):
  1. Offset conv for the group as 27 shifted fp16 matmuls (PE).
  2. p-pipeline on DVE: clip, floor, fracs, row index r=(d*35+h)*35+w.
  3. Fold indices to the 16-partition-wrapped int16 layout dma_gather needs
     (DRAM bounce, split per group so it overlaps the gather stream).
  4. ONE dma_gather per chunk: 3456 samples x 512B, each spanning two
     256B table rows (w-corner pair trick halves the table).
  5. Trilinear lerp on DVE (w, d, h stages), fp16, frac factors pre-expanded
     on the Scalar (ACT) engine so DVE runs in 2x perf mode.
  6. Contraction over (n, c) as 7 accumulated PE matmuls -> out[64, v].
Table: one 256B row per padded voxel = 4 (d,h)-corners x 32 ch fp16,
row-contiguous in DRAM; built with 4 xbar DMA transposes.
"""
import os
import sys

for _p in ('/opt/trn_rl_repo', '/root/.axon_site/_ro/trn_rl_repo'):
    if os.path.isdir(_p) and _p not in sys.path:
        sys.path.insert(0, _p)

import numpy as np
import ml_dtypes  # noqa

import concourse.bass as bass
import concourse.mybir as mybir
import concourse.tile as tile
from concourse import bacc
from concourse.bass_utils import run_bass_kernel_spmd
from concourse.masks import make_identity

F32 = mybir.dt.float32
F16 = mybir.dt.float16
I32 = mybir.dt.int32
I16 = mybir.dt.int16
AL = mybir.AluOpType

# ---------------- problem constants ----------------
C = 32          # input channels
O = 64          # output channels
NN = 27         # kernel sample points
NCORES = 8
DSH = 4         # d-planes per core
V = DSH * 32 * 32   # voxels per core = 4096
P35 = 35
PL = 16         # XE d-planes per core
PLSZ = P35 * P35    # 1225
XE_ROWS = PL * PLSZ  # 19600
TROWS = 19712        # 154 * 128 (padded table rows)
GRPS = TROWS // 128  # 154
XE_FREE = 22400      # >= TROWS + max shift (1261)
NVC = 32             # voxel chunks of 128
NG = 8               # pipeline groups (4 chunks each)
NI = NN * 128        # gather indices per chunk

_PROGRAM_CACHE = {}


def _build_program():
    nc = bacc.Bacc("TRN2", target_bir_lowering=False, debug=False)

    xe_d = nc.dram_tensor("xe", [C, XE_FREE], F16, kind="ExternalInput").ap()
    pc_d = nc.dram_tensor("pc", [128, NVC * 96], F32, kind="ExternalInput").ap()
    dclip_d = nc.dram_tensor("dclip", [128, 2], F32, kind="ExternalInput").ap()
    wp_d = nc.dram_tensor("wp", [32, NN * 96], F16, kind="ExternalInput").ap()
    wd_d = nc.dram_tensor("wd", [128, 7 * O], F16, kind="ExternalInput").ap()
    out_d = nc.dram_tensor("out_sl", [O, V], F32, kind="ExternalOutput").ap()
    t_d = nc.dram_tensor("tdram", [TROWS, 128], F16).ap()
    wrd = nc.dram_tensor("wrdram", [16, NVC, NN, 8], I16).ap()

    with tile.TileContext(nc) as tc:
        with tc.tile_pool(name="const", bufs=1) as cpool:
            ident = cpool.tile([128, 128], F32)
            make_identity(nc, ident[:])
            wp_sb = cpool.tile([32, NN * 96], F16)
            nc.sync.dma_start(wp_sb[:], wp_d)
            wd_sb = cpool.tile([128, 7 * O], F16)
            nc.sync.dma_start(wd_sb[:], wd_d)
            pc_sb = cpool.tile([128, NVC * 96], F32)
            nc.sync.dma_start(pc_sb[:], pc_d)
            dclip_sb = cpool.tile([128, 2], F32)
            nc.sync.dma_start(dclip_sb[:], dclip_d)

            frac_t = cpool.tile([128, NVC * 96], F16)
            p_t = cpool.tile([128, NVC * 96], F32)
            wr_groups = []
            for _wi in range(NG):
                _t = cpool.tile([128, 1024], I16, tag=f"wrg{_wi}")
                wr_groups.append(_t)

            with tc.tile_pool(name="xe", bufs=1) as xepool:
                xe4 = xepool.tile([32, XE_FREE], F16)
                nc.sync.dma_start(xe4[:, :], xe_d)

                # ---------- corner-block table (4 dh-corners per 256B row) ----------
                with tc.tile_pool(name="tbl", bufs=1) as tpool:
                    t_sb = tpool.tile([128, GRPS * 128], F16, tag="tsb")
                    for ed in range(2):
                        for eh in range(2):
                            e = ed * 2 + eh
                            dlt = ed * PLSZ + eh * P35
                            src = xe4[0:32, dlt: dlt + GRPS * 128]
                            dst = t_sb[:].rearrange(
                                "p (g x) -> p g x", x=128
                            )[:, :, e * 32:(e + 1) * 32]
                            nc.sync.dma_start_transpose(out=dst, in_=src)
                    # DRAM row r = g*128 + p  (voxel-contiguous rows)
                    nc.sync.dma_start(
                        out=t_d.rearrange("(G p) x -> p G x", p=128),
                        in_=t_sb[:].rearrange("p (g x) -> p g x", x=128))

                with (
                    tc.tile_pool(name="convps", bufs=3, space="PSUM") as cps,
                    tc.tile_pool(name="trps", bufs=2, space="PSUM") as tps,
                    tc.tile_pool(name="ops", bufs=2, space="PSUM") as ops,
                    tc.tile_pool(name="pipe", bufs=2) as pipe,
                    tc.tile_pool(name="gat", bufs=3) as gpool,
                    tc.tile_pool(name="lerp", bufs=2) as lpool,
                    tc.tile_pool(name="fx", bufs=2) as fpool,
                    tc.tile_pool(name="accp", bufs=3) as apool,
                    tc.tile_pool(name="outp", bufs=4) as opool,
                ):
                    # ---------- head: conv + p-pipe + idx fold, per group ----------
                    for g in range(NG):
                        dl, hh = g // 2, g % 2
                        psc = cps.tile([96, 512], F32, tag="convps")
                        for k in range(NN):
                            kd, kh, kw = k // 9, (k // 3) % 3, k % 3
                            b0 = (dl + kd + 5) * PLSZ + (hh * 16 + kh) * P35 + kw
                            rhs = xe4[:, b0:b0 + 16 * P35].rearrange(
                                "p (a b) -> p a b", b=P35)[:, :, 0:32]
                            nc.tensor.matmul(
                                psc[:, :],
                                lhsT=wp_sb[0:32, k * 96:(k + 1) * 96],
                                rhs=rhs,
                                start=(k == 0),
                                stop=(k == NN - 1),
                            )
                        offg = pipe.tile([96, 512], F32, tag="offg")
                        nc.scalar.copy(offg[:, :], psc[:, :])

                        for c4 in range(4):
                            ch = g * 4 + c4
                            ptp = tps.tile([128, 96], F32, tag="trps")
                            nc.tensor.transpose(
                                ptp[:, :],
                                offg[:, c4 * 128:(c4 + 1) * 128],
                                ident[0:96, 0:96],
                            )
                            nc.vector.tensor_add(
                                p_t[:, ch * 96:(ch + 1) * 96], ptp[:, :],
                                pc_sb[:, ch * 96:(ch + 1) * 96])

                        # p-pipeline for this group's 4 chunks (384 cols)
                        c0 = g * 4 * 96
                        pg = p_t[:, c0:c0 + 384]
                        dv = pg.rearrange("p (v x) -> p v x", x=96)[:, :, 0:27]
                        hwv = pg.rearrange("p (v x) -> p v x", x=96)[:, :, 32:91]
                        nc.vector.scalar_tensor_tensor(
                            out=dv, in0=dv, scalar=dclip_sb[:, 0:1],
                            in1=dclip_sb[:, 1:2].rearrange(
                                "p (a b) -> p a b", b=1).to_broadcast((128, 4, 27)),
                            op0=AL.max, op1=AL.min)
                        nc.vector.tensor_scalar(
                            out=hwv, in0=hwv, scalar1=0.0, scalar2=33.0,
                            op0=AL.max, op1=AL.min)

                        q0i = pipe.tile([128, 384], I32, tag="q0i")
                        nc.vector.tensor_copy(q0i[:], pg)
                        q0f = pipe.tile([128, 384], F32, tag="q0f")
                        nc.vector.tensor_copy(q0f[:], q0i[:])
                        fixt = pipe.tile([128, 384], F32, tag="fixt")
                        nc.vector.tensor_tensor(out=fixt[:], in0=q0f[:], in1=pg,
                                                op=AL.is_gt)
                        nc.vector.tensor_sub(q0f[:], q0f[:], fixt[:])
                        nc.vector.tensor_sub(frac_t[:, c0:c0 + 384], pg, q0f[:])
                        q0dv = q0f[:].rearrange("p (v x) -> p v x", x=96)[:, :, 0:27]
                        nc.vector.tensor_scalar(
                            out=q0dv, in0=q0dv, scalar1=0.0, scalar2=14.0,
                            op0=AL.max, op1=AL.min)

                        q0hv = q0f[:].rearrange("p (v x) -> p v x", x=96)[:, :, 32:59]
                        q0wv = q0f[:].rearrange("p (v x) -> p v x", x=96)[:, :, 64:91]
                        idxf = pipe.tile([128, 4 * 27], F32, tag="idxf")
                        iv = idxf[:].rearrange("p (v x) -> p v x", x=27)
                        nc.vector.scalar_tensor_tensor(
                            out=iv, in0=q0dv, scalar=35.0, in1=q0hv,
                            op0=AL.mult, op1=AL.add)
                        nc.vector.scalar_tensor_tensor(
                            out=iv, in0=iv, scalar=35.0, in1=q0wv,
                            op0=AL.mult, op1=AL.add)
                        idx16 = pipe.tile([128, 4 * 27], I16, tag="idx16")
                        nc.vector.tensor_copy(idx16[:], idxf[:])

                        # fold to wrapped layout via DRAM bounce (per group):
                        # wrg[q, vc, 8j + r] = idx16[16r + q, vc, j]
                        wrg = wr_groups[g]
                        for r in range(8):
                            out_v = wrd[:, g * 4:(g + 1) * 4, :, r:r + 1].rearrange(
                                "q vc j u -> q vc (u j)")
                            in_v = idx16[16 * r:16 * (r + 1), :].rearrange(
                                "q (vc j) -> q vc j", vc=4, j=NN)
                            nc.sync.dma_start(out=out_v, in_=in_v)
                        nc.sync.dma_start(
                            out=wrg[0:16, :].rearrange(
                                "q (vc x) -> q vc x", x=256)[:, :, 0:NN * 8],
                            in_=wrd[:, g * 4:(g + 1) * 4, :, :].rearrange(
                                "q vc j r8 -> q vc (j r8)"))
                        nc.sync.dma_start(out=wrg[16:32, :], in_=wrg[0:16, :])
                        nc.sync.dma_start(out=wrg[32:64, :], in_=wrg[0:32, :])
                        nc.sync.dma_start(out=wrg[64:128, :], in_=wrg[0:64, :])

                    # ---------- main loop: gather + lerp + contract per chunk ----------
                    gin_ap = bass.AP(t_d.tensor, 0, [[128, TROWS - 2], [1, 256]])
                    for vc in range(NVC):
                        rt = gpool.tile([128, NN * 256], F16, tag="rt")
                        nc.gpsimd.dma_gather(
                            out_ap=rt[:].rearrange("p (g x) -> p g x", x=256),
                            in_ap=gin_ap,
                            idxs_ap=wr_groups[vc // 4][
                                :, (vc % 4) * 256:(vc % 4) * 256 + NN * 8],
                            num_idxs=NI,
                            num_idxs_reg=NI,
                            elem_size=256,
                            elem_step=128,
                            single_packet=False,
                        )
                        rv = rt[:].rearrange("p (n x) -> p n x", x=256)

                        # expand frac factors to full width on ACT (2x DVE mode)
                        def _fexp(col, rep, tag):
                            fx = fpool.tile([128, NN * rep], F16, tag=tag)
                            fxv = fx[:].rearrange("p (n x) -> p n x", x=rep)
                            s = frac_t[:, vc * 96 + col: vc * 96 + col + 27]
                            nc.scalar.copy(
                                out=fxv,
                                in_=s.rearrange(
                                    "p (n o) -> p n o", o=1).to_broadcast(
                                    (128, NN, rep)))
                            return fxv
                        fw = _fexp(64, 128, "fw")
                        fd = _fexp(0, 64, "fd")
                        fh = _fexp(32, 32, "fh")

                        # rt row layout: [w-pair(2) x ed(2) x eh(2) x c(32)]
                        d1 = lpool.tile([128, NN * 128], F16, tag="d1")
                        av = d1[:].rearrange("p (n x) -> p n x", x=128)
                        nc.vector.tensor_sub(av, rv[:, :, 128:256], rv[:, :, 0:128])
                        nc.vector.tensor_tensor(out=av, in0=av, in1=fw, op=AL.mult)
                        nc.vector.tensor_add(av, av, rv[:, :, 0:128])

                        b1 = lpool.tile([128, NN * 64], F16, tag="b1")
                        bv = b1[:].rearrange("p (n x) -> p n x", x=64)
                        nc.vector.tensor_sub(bv, av[:, :, 64:128], av[:, :, 0:64])
                        nc.vector.tensor_tensor(out=bv, in0=bv, in1=fd, op=AL.mult)
                        nc.vector.tensor_add(bv, bv, av[:, :, 0:64])

                        acc = apool.tile([128, 896], F16, tag="acc")
                        nc.vector.memset(acc[:, NN * 32:896], 0.0)
                        cv = acc[:, 0:NN * 32].rearrange("p (n x) -> p n x", x=32)
                        nc.vector.tensor_sub(cv, bv[:, :, 32:64], bv[:, :, 0:32])
                        nc.vector.tensor_tensor(out=cv, in0=cv, in1=fh, op=AL.mult)
                        nc.vector.tensor_add(cv, cv, bv[:, :, 0:32])

                        acct = gpool.tile([128, 7, 128], F16, tag="acct")
                        nc.sync.dma_start_transpose(out=acct[:], in_=acc[:, :])

                        pso = ops.tile([64, 128], F32, tag="pso")
                        for gg in range(7):
                            nc.tensor.matmul(
                                pso[:, :],
                                lhsT=wd_sb[:, gg * O:(gg + 1) * O],
                                rhs=acct[:, gg, :],
                                start=(gg == 0), stop=(gg == 6))
                        osb = opool.tile([64, 128], F32, tag="osb")
                        nc.scalar.copy(osb[:], pso[:, :])
                        nc.sync.dma_start(
                            out=out_d[:, vc * 128:(vc + 1) * 128], in_=osb[:])

    nc.compile()
    return nc


def _host_prep(x, w_p, b_p, w_d):
    """Build per-core input maps."""
    x = np.asarray(x, np.float32)
    w_p = np.asarray(w_p, np.float32)
    b_p = np.asarray(b_p, np.float32)
    w_d = np.asarray(w_d, np.float32)

    # global padded/extended volume, channel-first, fp16:
    # XG[c, g, h', w'] with g = xp_plane + 5 (xp planes -5..39), h', w' in [0,35)
    XG = np.zeros((C, 45, P35, P35), np.float16)
    XG[:, 6:38, 1:33, 1:33] = x[0].astype(np.float16)

    # pc (shared): [128, 32*96] f32
    v = np.arange(V)
    dl, hh, wl = v >> 10, (v >> 5) & 31, v & 31
    r = np.array([-1.0, 0.0, 1.0], np.float32)
    pn_d, pn_h, pn_w = np.meshgrid(r, r, r, indexing='ij')
    pn = np.stack([pn_d.ravel(), pn_h.ravel(), pn_w.ravel()])  # (3, 27)
    pc = np.zeros((V, 96), np.float32)
    pc[:, 0:27] = (dl[:, None] + 6.0) + pn[0][None, :] + b_p[None, 0:27]
    pc[:, 32:59] = (hh[:, None] + 1.0) + pn[1][None, :] + b_p[None, 27:54]
    pc[:, 64:91] = (wl[:, None] + 1.0) + pn[2][None, :] + b_p[None, 54:81]
    pc_t = pc.reshape(NVC, 128, 96).transpose(1, 0, 2).reshape(128, NVC * 96)
    pc_t = np.ascontiguousarray(pc_t, np.float32)

    # wp lhsT: [32, 27*96] fp16 (one 96-col slice per kernel tap)
    wp_l = np.zeros((32, NN * 96), np.float16)
    colmap = np.full(96, -1, np.int64)
    colmap[0:27] = np.arange(27)
    colmap[32:59] = 27 + np.arange(27)
    colmap[64:91] = 54 + np.arange(27)
    for k in range(NN):
        kd, kh, kw = k // 9, (k // 3) % 3, k % 3
        for m in range(96):
            ch = colmap[m]
            if ch < 0:
                continue
            wp_l[:, k * 96 + m] = w_p[ch, :, kd, kh, kw]

    # wd lhsT: [128, 7*64] fp16. K-row layout must match acc cols (n*32+c):
    # K = g*128 + pk -> n = (g*128+pk)//32, c = pk%32
    wd_l = np.zeros((128, 7 * O), np.float16)
    for g in range(7):
        for pk in range(128):
            n = 4 * g + pk // 32
            if n >= NN:
                continue
            wd_l[pk, g * O:(g + 1) * O] = w_d[:, pk % 32, n // 9, (n // 3) % 3, n % 3]

    in_maps = []
    for k in range(NCORES):
        dlo = 4 * k - 5
        xe = np.zeros((C, XE_FREE), np.float16)
        xe[:, :XE_ROWS] = XG[:, 4 * k:4 * k + PL].reshape(C, XE_ROWS)
        dclip = np.zeros((128, 2), np.float32)
        dclip[:, 0] = 0.0 - dlo
        dclip[:, 1] = 33.0 - dlo
        in_maps.append({
            "xe": xe,
            "pc": pc_t,
            "dclip": dclip,
            "wp": wp_l,
            "wd": wd_l,
        })
    return in_maps


def kernel(x, w_p, b_p, w_d):
    if "nc" not in _PROGRAM_CACHE:
        _PROGRAM_CACHE["nc"] = _build_program()
    nc = _PROGRAM_CACHE["nc"]
    in_maps = _host_prep(x, w_p, b_p, w_d)
    res = run_bass_kernel_spmd(nc, in_maps, list(range(NCORES))).results
    out = np.empty((1, O, 32, 32, 32), np.float32)
    for k in range(NCORES):
        out[0, :, 4 * k:4 * k + 4] = res[k]["out_sl"].reshape(O, DSH, 32, 32)
    return out
